# revision 22
# baseline (speedup 1.0000x reference)
"""Trainium2 Bass kernel for a DGL-style digit-capsule routing layer.

Inputs (full, unsharded):
    x      [256, 8, 1152] f32   -- B, D_IN, N_IN
    weight [1152, 10, 16, 8] f32 -- N_IN, N_OUT, D_OUT, D_IN
Output:
    v      [256, 10, 16] f32

Algorithm (exact refactor of the reference, never materializing u_hat):
    s[b,(j,o)]    = sum_{(i,d)} c[i,j] W[(i,d),(j,o)] x[b,(i,d)]     (matmul over (i,d))
    v             = squash(s)
    b_upd[i,j]    = (1/B) sum_d sum_o W[(i,d),(j,o)] M[(i,d),(j,o)]
      where M[(i,d),(j,o)] = sum_b x[b,(i,d)] v[b,(j,o)]             (matmul over b)

Sharding: input capsules i are split 8 ways (144 per core). Per routing
iteration the only cross-core data is the partial sum of s ([256,160],
carried in fp16): iterations 1-2 use AllReduce, iteration 3 uses
ReduceScatter so each core squashes and emits its own 32-row shard of
the final output.

Perf notes vs the v1 kernel:
  - all collectives + their DMA hops moved from fp32 to fp16 (half bytes)
  - the agreement update keeps M in the [(i,d),(j,o)] orientation so the
    d-sum becomes ONE matmul with a constant block-diagonal stationary
    operand (replaces 18 small selector matmuls + two strided reduces
    + broadcast copies)
  - squash's rsqrt is computed on the vector engine via the sqrt-bits
    trick + reciprocal + one Newton step, so the scalar engine only ever
    runs Exp and never reloads activation tables
  - Wc is written per 160-col chunk so the next s-matmul chunk can start
    as soon as its chunk is ready
"""

import numpy as np

N_CORES = 8
B = 256
NI, NO, DO, DI = 1152, 10, 16, 8
JO = NO * DO            # 160
IL = NI // N_CORES      # 144 capsules per core
ID = IL * DI            # 1152 (i,d) rows per core
NCH = ID // 128         # 9 partition chunks
BL = B // N_CORES       # 32 output batch rows per core
SQRT_MAGIC = 0x1FBD1DF5  # bits(sqrt(x)) ~= (bits(x)>>1) + MAGIC

_STATE = {}


def _build(repeat=1):
    """Build the bass program. ``repeat`` > 1 duplicates the full routing
    computation (for slope-based HW timing); the output is unchanged."""
    import concourse.bass as bass
    import concourse.bacc as bacc
    import concourse.mybir as mybir
    import concourse.tile as tile

    dt = mybir.dt
    AF = mybir.ActivationFunctionType
    ALU = mybir.AluOpType

    nc = bacc.Bacc(None, num_devices=N_CORES)

    # Per-core external inputs (pre-sharded/pre-laid-out on host).
    xr = nc.declare_dram_parameter("xr", [128, NCH * B], dt.float16, isOutput=False)
    wt = nc.declare_dram_parameter("wt", [128, NCH * JO], dt.float16, isOutput=False)
    xid = nc.declare_dram_parameter("xid", [128, 2 * ID], dt.float16, isOutput=False)
    amat = nc.declare_dram_parameter("amat", [128, 128], dt.float16, isOutput=False)
    v_out = nc.declare_dram_parameter("v_out", [BL // 2, 2 * JO], dt.float32,
                                      isOutput=True)

    # Internal DRAM bounce buffers for the collectives (per repeat x iter).
    # Iterations 0/1 use one AllReduce per 128-row batch half (kb) so the
    # squash + agreement-matmul work for half 0 hides under half 1's
    # AllReduce; iteration 2 uses a single joint ReduceScatter.
    # Collective payloads live as [128, 2*JO]: batch half kb sits in the
    # column range [kb*JO, (kb+1)*JO) so each hop is ONE contiguous DMA.
    cc_in_all, cc_out_all = [], []
    for rep in range(repeat):
        cc_in_all.append(
            [nc.dram_tensor(f"cc_in{rep}_{t}", [128, 2 * JO], dt.float16)
             for t in range(3)])
        cc_out_all.append([
            nc.dram_tensor(f"cc_out{rep}_0", [128, 2 * JO], dt.float16,
                           addr_space="Shared"),
            nc.dram_tensor(f"cc_out{rep}_1", [128, 2 * JO], dt.float16,
                           addr_space="Shared"),
            nc.dram_tensor(f"cc_out{rep}_2", [BL // 2, 2 * JO], dt.float16),
        ])
    groups = [list(range(N_CORES))]

    with tile.TileContext(nc) as tc:
        with tc.tile_pool(name="const", bufs=1) as cpool, \
             tc.tile_pool(name="work", bufs=2) as wpool, \
             tc.tile_pool(name="sq", bufs=2) as qpool, \
             tc.tile_pool(name="psum_s", bufs=2, space="PSUM") as ps_pool, \
             tc.tile_pool(name="psum_m", bufs=1, space="PSUM") as pm_pool, \
             tc.tile_pool(name="psum_b", bufs=2, space="PSUM") as pb_pool:

            # ---- constant/persistent tiles ----
            xr_t = cpool.tile([128, NCH * B], dt.float16, tag="xr")
            wt_t = cpool.tile([128, NCH * JO], dt.float16, tag="wt")
            xid_t = cpool.tile([128, 2 * ID], dt.float16, tag="xid")
            amat_t = cpool.tile([128, 128], dt.float16, tag="amat")
            b_ch = cpool.tile([128, NCH * NO], dt.float32, tag="bch")
            wc_t = cpool.tile([128, NCH * JO], dt.float16, tag="wc")

            nc.sync.dma_start(xr_t[:], xr[:])
            nc.sync.dma_start(wt_t[:], wt[:])
            nc.sync.dma_start(xid_t[:], xid[:])
            nc.sync.dma_start(amat_t[:], amat[:])

            def squash(sf, p, out_dt, tag, ng=NO):
                """v = s * sq/(1+sq)/sqrt(sq); rsqrt via sqrt-bits trick
                + reciprocal + one Newton step, all on the vector engine.
                ``ng`` capsule groups of DO columns are squashed at once."""
                w = ng * DO
                t2 = wpool.tile([p, w], dt.float32, tag=f"ssq{tag}")
                nc.vector.tensor_mul(t2[:], sf[:], sf[:])
                sq = qpool.tile([p, ng], dt.float32, tag=f"sq{tag}")
                nc.vector.reduce_sum(
                    out=sq[:], in_=t2[:].rearrange("p (j o) -> p j o", j=ng),
                    axis=mybir.AxisListType.X)
                sb = qpool.tile([p, ng], dt.float32, tag=f"sb{tag}")
                nc.vector.tensor_scalar(
                    sb[:].bitcast(dt.uint32), sq[:].bitcast(dt.uint32),
                    1, None, ALU.logical_shift_right)
                nc.vector.tensor_scalar(
                    sb[:].bitcast(dt.uint32), sb[:].bitcast(dt.uint32),
                    SQRT_MAGIC, None, ALU.add)
                y0 = qpool.tile([p, ng], dt.float32, tag=f"y0{tag}")
                nc.vector.reciprocal(y0[:], sb[:])
                a = qpool.tile([p, ng], dt.float32, tag=f"a{tag}")
                nc.vector.tensor_mul(a[:], sq[:], y0[:])
                nc.vector.tensor_mul(a[:], a[:], y0[:])
                nw = qpool.tile([p, ng], dt.float32, tag=f"nw{tag}")
                nc.vector.tensor_scalar(
                    nw[:], a[:], -0.5, 1.5, ALU.mult, ALU.add)
                # fac = sq*y0*nw / (1+sq)
                r1 = qpool.tile([p, ng], dt.float32, tag=f"r1{tag}")
                nc.vector.tensor_scalar_add(r1[:], sq[:], 1.0)
                rd = qpool.tile([p, ng], dt.float32, tag=f"rd{tag}")
                nc.vector.reciprocal(rd[:], r1[:])
                f1 = qpool.tile([p, ng], dt.float32, tag=f"f1{tag}")
                nc.vector.tensor_mul(f1[:], sq[:], rd[:])
                nc.vector.tensor_mul(f1[:], f1[:], y0[:])
                nc.vector.tensor_mul(f1[:], f1[:], nw[:])
                vt = wpool.tile([p, w], out_dt, tag=f"v{tag}")
                nc.vector.tensor_tensor(
                    out=vt[:].rearrange("p (j o) -> p j o", j=ng),
                    in0=sf[:].rearrange("p (j o) -> p j o", j=ng),
                    in1=f1[:].unsqueeze(2).to_broadcast((p, ng, DO)),
                    op=ALU.mult)
                return vt

            for rep in range(repeat):
              cc_in = cc_in_all[rep]
              cc_out = cc_out_all[rep]
              for t in range(3):
                # ---- s matmul: psum_s[kb] = sum_ci xr[:,ci,kb]^T @ w ----
                rhs_w = wt_t if t == 0 else wc_t
                st = wpool.tile([128, 2 * JO], dt.float16, tag="s_sb")
                for kb in range(2):
                    ps = ps_pool.tile([128, JO], dt.float32, tag="ps")
                    for ci in range(NCH):
                        lhs = xr_t[:, ci * B + kb * 128: ci * B + kb * 128 + 128]
                        rhs = rhs_w[:, ci * JO:(ci + 1) * JO]
                        nc.tensor.matmul(ps[:], lhs, rhs,
                                         start=(ci == 0), stop=(ci == NCH - 1))
                    # PSUM->fp16 convert on the (otherwise idle) scalar engine;
                    # c == 1/10 exactly on iteration 1: fold into the copy
                    stk = st[:, kb * JO:(kb + 1) * JO]
                    if t == 0:
                        nc.scalar.mul(stk, ps[:], 0.1)
                    else:
                        nc.scalar.copy(stk, ps[:])
                    # store each half as soon as its copy lands; half 0's
                    # store overlaps half 1's matmuls
                    nc.sync.dma_start(cc_in[t][:, kb * JO:(kb + 1) * JO], stk)

                if t == 2:
                    nc.gpsimd.collective_compute(
                        "ReduceScatter", ALU.add, replica_groups=groups,
                        ins=[cc_in[2][:]], outs=[cc_out[2][:]],
                    )
                    sf = wpool.tile([BL // 2, 2 * JO], dt.float16, tag="s_rs")
                    nc.sync.dma_start(sf[:], cc_out[2][:])
                    v3 = squash(sf, BL // 2, dt.float32, 3, ng=2 * NO)
                    nc.sync.dma_start(v_out[:], v3[:])
                    break

                nc.gpsimd.collective_compute(
                    "AllReduce", ALU.add, replica_groups=groups,
                    ins=[cc_in[t][:]], outs=[cc_out[t][:]],
                )

                # ---- squash both batch halves in one chain ----
                sf2 = wpool.tile([128, 2 * JO], dt.float16, tag="s_full")
                nc.sync.dma_start(sf2[:], cc_out[t][:])
                v2t = squash(sf2, 128, dt.float16, "b", ng=2 * NO)

                # ---- agreement matmuls: M[(i,d),(j,o)] = sum_b x v ----
                qt = wpool.tile([128, NCH * NO], dt.float16, tag="qt")
                pms = []
                for g in range(3):
                    # 3 chunks share one 2KB PSUM bank (3 x 160 f32 cols)
                    pmg = pm_pool.tile([128, 3 * JO], dt.float32, tag=f"pm{g}")
                    pms.append(pmg)
                for ci in range(NCH):
                    # stride banks so consecutive chunks never share a bank
                    pm = pms[ci % 3][:, (ci // 3) * JO:(ci // 3 + 1) * JO]
                    for kb in range(2):
                        lhs = xid_t[:, kb * ID + ci * 128: kb * ID + ci * 128 + 128]
                        nc.tensor.matmul(pm, lhs, v2t[:, kb * JO:(kb + 1) * JO],
                                         start=(kb == 0), stop=(kb == 1))
                    pt = wpool.tile([128, JO], dt.float16, tag="pt")
                    nc.vector.tensor_mul(
                        pt[:], pm, wt_t[:, ci * JO:(ci + 1) * JO])
                    with nc.allow_low_precision("o-sum, fp16"):
                        nc.vector.reduce_sum(
                            out=qt[:, ci * NO:(ci + 1) * NO],
                            in_=pt[:].rearrange("p (j o) -> p j o", j=NO),
                            axis=mybir.AxisListType.X)

                # ---- b_upd: one matmul with constant block-diag A sums d ----
                pb = pb_pool.tile([128, NCH * NO], dt.float32, tag="pb")
                nc.tensor.matmul(pb[:], amat_t[:], qt[:], start=True, stop=True)

                if t == 0:
                    nc.vector.tensor_copy(b_ch[:], pb[:])
                else:
                    nc.vector.tensor_add(b_ch[:], b_ch[:], pb[:])

                # ---- softmax over j (free dim within each chunk) ----
                e_ch = wpool.tile([128, NCH * NO], dt.float32, tag="ech")
                nc.scalar.activation(e_ch[:], b_ch[:], AF.Exp)
                z_ch = wpool.tile([128, NCH], dt.float32, tag="zch")
                nc.vector.reduce_sum(
                    out=z_ch[:], in_=e_ch[:].rearrange("p (c j) -> p c j", c=NCH),
                    axis=mybir.AxisListType.X)
                r_ch = wpool.tile([128, NCH], dt.float32, tag="rch")
                nc.vector.reciprocal(r_ch[:], z_ch[:])
                c_ch = wpool.tile([128, NCH * NO], dt.float32, tag="cch")
                nc.vector.tensor_tensor(
                    out=c_ch[:].rearrange("p (c j) -> p c j", c=NCH),
                    in0=e_ch[:].rearrange("p (c j) -> p c j", c=NCH),
                    in1=r_ch[:].unsqueeze(2).to_broadcast((128, NCH, NO)),
                    op=ALU.mult)

                # ---- Wc = Wt * c (fp16), per chunk so s-matmuls can pipeline ----
                for ci in range(NCH):
                    nc.vector.tensor_tensor(
                        out=wc_t[:, ci * JO:(ci + 1) * JO]
                            .rearrange("p (j o) -> p j o", j=NO),
                        in0=wt_t[:, ci * JO:(ci + 1) * JO]
                            .rearrange("p (j o) -> p j o", j=NO),
                        in1=c_ch[:, ci * NO:(ci + 1) * NO]
                            .unsqueeze(2).to_broadcast((128, NO, DO)),
                        op=ALU.mult)

    return nc


def _get_runner():
    if "runner" in _STATE:
        return _STATE["runner"]

    import jax
    import numpy as np
    from concourse import bass2jax
    from concourse.bass2jax import (
        _bass_exec_p, install_neuronx_cc_hook, partition_id_tensor)
    from jax.experimental.shard_map import shard_map
    from jax.sharding import Mesh, PartitionSpec
    import concourse.mybir as mybir

    nc = _build()
    if not nc.is_finalized():
        nc.finalize()
    install_neuronx_cc_hook()

    partition_name = nc.partition_id_tensor.name if nc.partition_id_tensor else None
    in_names, out_names, out_avals, zero_outs = [], [], [], []
    for alloc in nc.m.functions[0].allocations:
        if not isinstance(alloc, mybir.MemoryLocationSet):
            continue
        name = alloc.memorylocations[0].name
        if alloc.kind == "ExternalInput":
            if name != partition_name:
                in_names.append(name)
        elif alloc.kind == "ExternalOutput":
            out_names.append(name)
            shape = tuple(alloc.tensor_shape)
            dtype = mybir.dt.np(alloc.dtype)
            out_avals.append(jax.core.ShapedArray(shape, dtype))
            zero_outs.append(np.zeros(shape, dtype))
    n_params = len(in_names)
    n_outs = len(out_avals)
    all_names = in_names + out_names
    if partition_name is not None:
        all_names = all_names + [partition_name]

    def _body(*args):
        operands = list(args)
        if partition_name is not None:
            operands.append(partition_id_tensor())
        outs = _bass_exec_p.bind(
            *operands,
            out_avals=tuple(out_avals),
            in_names=tuple(all_names),
            out_names=tuple(out_names),
            lowering_input_output_aliases=(),
            sim_require_finite=True,
            sim_require_nnan=True,
            nc=nc,
        )
        return tuple(outs)

    devices = jax.devices()[:N_CORES]
    assert len(devices) == N_CORES, f"need {N_CORES} cores, have {len(devices)}"
    mesh = Mesh(np.asarray(devices), ("core",))
    donate = tuple(range(n_params, n_params + n_outs))
    sharded = jax.jit(
        shard_map(_body, mesh=mesh,
                  in_specs=(PartitionSpec("core"),) * (n_params + n_outs),
                  out_specs=(PartitionSpec("core"),) * n_outs,
                  check_rep=False),
        donate_argnums=donate, keep_unused=True)

    runner = (sharded, in_names, out_names, [z.shape for z in zero_outs],
              [z.dtype for z in zero_outs])
    _STATE["runner"] = runner
    _STATE["nc"] = nc
    return runner


def _prep_core_inputs(x, weight, k):
    """Host-side slicing/layout for core k (i-shard of 144 capsules)."""
    i0, i1 = k * IL, (k + 1) * IL
    xs = np.ascontiguousarray(x[:, :, i0:i1])          # [256, 8, 144]
    ws = np.ascontiguousarray(weight[i0:i1])           # [144, 10, 16, 8]

    # [(i,d), b] i-major rows, then partition-chunked to [128, 9*256]
    xr = xs.transpose(2, 1, 0).reshape(ID, B)
    xr_ch = xr.reshape(NCH, 128, B).transpose(1, 0, 2).reshape(128, NCH * B)
    # [b, (i,d)] i-major cols, b-chunked to [128, 2*1152]
    xid = xs.transpose(0, 2, 1).reshape(B, IL * DI)
    xid_ch = np.concatenate([xid[0:128], xid[128:256]], axis=1)
    # [(i,d), (j,o)] -> chunked [128, 9*160]
    wt = ws.transpose(0, 3, 1, 2).reshape(ID, JO)
    wt_ch = wt.reshape(NCH, 128, JO).transpose(1, 0, 2).reshape(128, NCH * JO)

    return {
        "xr": xr_ch.astype(np.float16),
        "wt": wt_ch.astype(np.float16),
        "xid": xid_ch.astype(np.float16),
        "amat": _STATE["amat"],
    }


def _selectors():
    if "amat" not in _STATE:
        # block-diagonal (i,d)->(i,d') d-sum matrix with the 1/B mean folded in
        p = np.arange(128)
        _STATE["amat"] = ((p[:, None] // DI == p[None, :] // DI)
                          .astype(np.float32) / B).astype(np.float16)


def kernel(x, weight):
    x = np.asarray(x, dtype=np.float32)
    weight = np.asarray(weight, dtype=np.float32)
    _selectors()
    sharded, in_names, out_names, out_shapes, out_dtypes = _get_runner()

    per_core = [_prep_core_inputs(x, weight, k) for k in range(N_CORES)]
    concat_in = [
        np.concatenate([per_core[c][nm] for c in range(N_CORES)], axis=0)
        for nm in in_names
    ]
    concat_zero = [
        np.zeros((N_CORES * s[0],) + tuple(s[1:]), d)
        for s, d in zip(out_shapes, out_dtypes)
    ]
    outs = sharded(*concat_in, *concat_zero)
    v = np.asarray(outs[out_names.index("v_out")])   # [8*16, 320]
    full = np.empty((B, JO), np.float32)
    hl = BL // 2
    for k in range(N_CORES):
        vk = v[k * hl:(k + 1) * hl]
        full[k * hl:(k + 1) * hl] = vk[:, :JO]
        full[128 + k * hl:128 + (k + 1) * hl] = vk[:, JO:]
    return full.reshape(B, NO, DO)
